# revision 1
# baseline (speedup 1.0000x reference)
"""BiAttention kernel for Trainium2 (Bass/Tile), 8-core data-parallel over batch.

Reference computation (per batch example):
    input_dot[l]  = input @ w_input                    [L]
    memory_dot[m] = memory @ w_memory                  [M]
    cross[l,m]    = (input * dot_scale) @ memory^T     [L,M]
    att = input_dot + memory_dot + cross
    att = where(mask_l | mask_m, -1e20, att)
    w1 = softmax_m(att); o1 = w1 @ memory
    w2 = softmax_l(max_m(att)); o2 = w2 @ input        [1,D]
    out = concat([input, o1, input*o1, o2*o1], -1)     [L,4D]

Sharding: batch 16 -> 2 examples per core across 8 cores; D-sized vectors
replicated. Each core runs an identical NEFF on its own slice.

Matmuls run in float32r (full PE rate). The BIR verifier requires engine
producers of f32r operands to emit f32r-typed outputs (DMA producers are
exempt), so operand tiles are typed float32r and constants (identity, ones)
are DMA'd from NEFF-inline DRAM tensors.
"""

import sys

sys.path.insert(0, "/opt/trn_rl_repo")

import numpy as np

import concourse.bass as bass
import concourse.tile as tile
from concourse import bacc, mybir
from concourse.bass import ds, ts
from concourse.bass_utils import run_bass_kernel_spmd

F32 = mybir.dt.float32
F32R = mybir.dt.float32r
U8 = mybir.dt.uint8
NEG = -1.0e20
P = 128


def _r(ap):
    return ap.bitcast(F32R)


def _f(ap):
    return ap.bitcast(F32)


def biattn_tile_kernel(tc, out_ap, inp_ap, mem_ap, msk_ap, w_in_ap, w_mem_ap,
                       dscale_ap, BPC, L, D, M):
    nc = tc.nc
    KD = D // P            # d-chunks (contraction tiles for score matmul)
    NLT = L // P           # l-tiles
    NMC = M // P           # m-chunks
    AC = min(512, M)       # att column chunk (PSUM bank limit for fp32)
    NAC = M // AC
    DC = min(512, D)       # output_one column chunk
    ND2 = D // DC
    X = mybir.AxisListType.X
    Exp = mybir.ActivationFunctionType.Exp
    Copy = mybir.ActivationFunctionType.Copy

    ident_dram = nc.inline_tensor(np.eye(P, dtype=np.float32), name="identconst")
    ones_dram = nc.inline_tensor(np.ones((1, P), dtype=np.float32), name="onesconst")

    import contextlib
    ctx = contextlib.ExitStack()
    with ctx:
        # --- pools ---
        consts = ctx.enter_context(tc.tile_pool(name="consts", bufs=1))
        residents = ctx.enter_context(tc.tile_pool(name="residents", bufs=1))
        inpool = ctx.enter_context(tc.tile_pool(name="inpool", bufs=2))
        sitpool = ctx.enter_context(tc.tile_pool(name="sitpool", bufs=1))
        att2pool = ctx.enter_context(tc.tile_pool(name="att2pool", bufs=2))
        ptpool = ctx.enter_context(tc.tile_pool(name="ptpool", bufs=1))
        o1pool = ctx.enter_context(tc.tile_pool(name="o1pool", bufs=2))
        b3pool = ctx.enter_context(tc.tile_pool(name="b3pool", bufs=2))
        smalls = ctx.enter_context(tc.tile_pool(name="smalls", bufs=2))
        attps = ctx.enter_context(tc.tile_pool(name="attps", bufs=2, space="PSUM"))
        tpps = ctx.enter_context(tc.tile_pool(name="tpps", bufs=2, space="PSUM"))
        o1ps = ctx.enter_context(tc.tile_pool(name="o1ps", bufs=1, space="PSUM"))
        o2ps = ctx.enter_context(tc.tile_pool(name="o2ps", bufs=1, space="PSUM"))
        drampool = ctx.enter_context(tc.tile_pool(name="drampool", bufs=1, space="DRAM"))

        # --- constants ---
        ident_r = consts.tile([P, P], F32R)     # for f32r-mode PE transposes
        nc.sync.dma_start(out=ident_r, in_=_r(ident_dram.ap()))
        ident_f = consts.tile([P, P], F32)      # for f32-mode PE transposes
        nc.sync.dma_start(out=ident_f, in_=ident_dram.ap())
        ones_r = consts.tile([1, P], F32R)      # K=1 stationary for extra row
        nc.sync.dma_start(out=ones_r, in_=_r(ones_dram.ap()))
        ones_f = consts.tile([P, 1], F32)       # fp32 reduction helper
        nc.vector.memset(ones_f, 1.0)
        neg30 = consts.tile([P, 1], F32)        # bias for shifted exp (o2 path)
        nc.vector.memset(neg30, -30.0)
        w_mem_t = consts.tile([P, KD], F32R)    # w_memory in d-major layout
        ds_t = consts.tile([P, KD], F32)        # dot_scale in d-major layout
        for k in range(KD):
            nc.sync.dma_start(out=w_mem_t[:, k:k + 1],
                              in_=_r(w_mem_ap[ts(k, P)].unsqueeze(-1)))
            nc.sync.dma_start(out=ds_t[:, k:k + 1],
                              in_=dscale_ap[ts(k, P)].unsqueeze(-1))
        w_in_bcast = consts.tile([P, D], F32)   # w_input replicated on partitions
        nc.sync.dma_start(
            out=w_in_bcast,
            in_=bass.AP(tensor=w_in_ap.tensor, offset=w_in_ap.offset,
                        ap=[[0, P]] + list(w_in_ap.ap)),
        )

        for b in range(BPC):
            # --- load memory resident, build memT via PE transposes (f32r) ---
            mem_big = residents.tile([P, NMC, D], F32R, tag="mem")
            for mc in range(NMC):
                nc.sync.dma_start(out=mem_big[:, mc, :],
                                  in_=_r(mem_ap[b, ts(mc, P), :]))
            memT = residents.tile([P, KD, M], F32R, tag="memT")
            for mc in range(NMC):
                for k in range(KD):
                    tp = tpps.tile([P, P], F32R, tag="tp")
                    nc.tensor.transpose(tp, mem_big[:, mc, ts(k, P)], ident_r)
                    nc.scalar.copy(out=memT[:, k, ds(mc * P, P)], in_=tp)

            # --- mask row + memory_dot -> extra_row (K=1 fused score row) ---
            mask_row = residents.tile([1, M], U8, tag="mrow")
            nc.sync.dma_start(out=mask_row, in_=msk_ap[b:b + 1, :])
            extra_row = residents.tile([1, M], F32R, tag="erow")
            for c in range(NAC):
                mdot_ps = attps.tile([1, AC], F32, tag="att")
                for k in range(KD):
                    nc.tensor.matmul(mdot_ps, w_mem_t[:, k:k + 1],
                                     memT[:, k, ds(c * AC, AC)],
                                     start=(k == 0), stop=(k == KD - 1))
                # extra_row = mask_m * NEG + memory_dot (exactly NEG if masked)
                mneg = residents.tile([1, AC], F32, tag="mneg")
                nc.vector.tensor_scalar(out=mneg,
                                        in0=mask_row[0:1, ds(c * AC, AC)],
                                        scalar1=NEG, scalar2=0.0,
                                        op0=mybir.AluOpType.mult,
                                        op1=mybir.AluOpType.add)
                nc.vector.tensor_add(out=extra_row[0:1, ds(c * AC, AC)],
                                     in0=mneg, in1=mdot_ps)

            # --- output_two accumulators ---
            nrm_all = residents.tile([P, NLT], F32, tag="nrmall")
            o2_ps = o2ps.tile([1, D], F32, tag="o2")

            for lt in range(NLT):
                lsl = ts(lt, P)
                in_t = inpool.tile([P, D], F32R, tag="in")
                nc.sync.dma_start(out=in_t, in_=_r(inp_ap[b, lsl, :]))
                mlt = smalls.tile([P, 1], U8, tag="mlt")
                nc.sync.dma_start(out=mlt, in_=msk_ap[b, lsl].unsqueeze(-1))

                # input_dot on DVE (w_input replicated across partitions)
                junk = o1pool.tile([P, D], F32, tag="o1")
                nc.vector.tensor_tensor(out=junk, in0=_f(in_t),
                                        in1=w_in_bcast,
                                        op=mybir.AluOpType.mult)
                idot = smalls.tile([P, 1], F32, tag="idot")
                nc.vector.reduce_sum(out=idot, in_=junk, axis=X)

                # s1 = 1-mask_l ; s2 = idot*(1-mask_l) + NEG*mask_l
                mask_f = smalls.tile([P, 1], F32, tag="maskf")
                nc.vector.tensor_copy(out=mask_f, in_=mlt)
                s1 = smalls.tile([P, 1], F32, tag="s1")
                nc.vector.tensor_scalar(out=s1, in0=mask_f, scalar1=-1.0,
                                        scalar2=1.0, op0=mybir.AluOpType.mult,
                                        op1=mybir.AluOpType.add)
                s2t = smalls.tile([P, 1], F32, tag="s2t")
                nc.vector.tensor_tensor(out=s2t, in0=idot, in1=s1,
                                        op=mybir.AluOpType.mult)
                s2m = smalls.tile([P, 1], F32, tag="s2m")
                nc.vector.tensor_scalar(out=s2m, in0=mask_f, scalar1=NEG,
                                        scalar2=0.0, op0=mybir.AluOpType.mult,
                                        op1=mybir.AluOpType.add)
                s2 = smalls.tile([P, 1], F32, tag="s2")
                nc.vector.tensor_add(out=s2, in0=s2m, in1=s2t)

                # scaled-input transpose: siT[d,l] = input^T * dot_scale
                siT = sitpool.tile([P, KD * P], F32R, tag="sit")
                for k in range(KD):
                    tp = tpps.tile([P, P], F32R, tag="tp")
                    nc.tensor.transpose(tp, in_t[:, ts(k, P)], ident_r)
                    nc.vector.tensor_scalar_mul(out=siT[:, ts(k, P)],
                                                in0=_f(tp),
                                                scalar1=ds_t[:, k:k + 1])

                # scores: cross + extra_row, then mask_l/input_dot fuse on DVE
                att2 = att2pool.tile([P, M], F32, tag="att2")
                cmax = smalls.tile([P, NAC], F32, tag="cmax")
                for c in range(NAC):
                    att_ps = attps.tile([P, AC], F32, tag="att")
                    for k in range(KD):
                        nc.tensor.matmul(att_ps, siT[:, ts(k, P)],
                                         memT[:, k, ds(c * AC, AC)],
                                         start=(k == 0), stop=False)
                    nc.tensor.matmul(att_ps, ones_r,
                                     extra_row[0:1, ds(c * AC, AC)],
                                     start=False, stop=True)
                    nc.vector.tensor_scalar(
                        out=att2[:, ds(c * AC, AC)], in0=att_ps, scalar1=s1,
                        scalar2=s2, op0=mybir.AluOpType.mult,
                        op1=mybir.AluOpType.add)
                    nc.vector.reduce_max(out=cmax[:, c:c + 1],
                                         in_=att2[:, ds(c * AC, AC)], axis=X)
                rmax = smalls.tile([P, 1], F32, tag="rmax")
                nc.vector.reduce_max(out=rmax, in_=cmax, axis=X)
                nrm = nrm_all[:, lt:lt + 1]                 # -rowmax
                nc.vector.tensor_scalar(out=nrm, in0=rmax, scalar1=-1.0,
                                        scalar2=0.0, op0=mybir.AluOpType.mult,
                                        op1=mybir.AluOpType.add)

                # output_two partials: ev = exp(rowmax - 30) (f32r for matmul)
                ev = smalls.tile([P, 1], F32R, tag="ev")
                nc.scalar.activation(out=ev, in_=nrm, func=Exp, scale=-1.0,
                                     bias=neg30)
                for dc in range(ND2):
                    nc.tensor.matmul(o2_ps[0:1, ds(dc * DC, DC)], ev,
                                     in_t[:, ds(dc * DC, DC)],
                                     start=(lt == 0), stop=(lt == NLT - 1))

                # P = exp(att - rowmax) in place, rowsum via ACT accumulator
                rowsum = smalls.tile([P, 1], F32, tag="rsum")
                nc.scalar.activation(out=att2, in_=att2, func=Exp, bias=nrm,
                                     scale=1.0, accum_out=rowsum)
                recip = smalls.tile([P, 1], F32, tag="recip")
                nc.vector.reciprocal(recip, rowsum)

                # transpose P -> PT (m-major, f32 mode; ACT copy rounds to f32r)
                PT = ptpool.tile([P, M], F32R, tag="pt")
                for mc in range(NMC):
                    tp = tpps.tile([P, P], F32, tag="tp")
                    nc.tensor.transpose(tp, att2[:, ts(mc, P)], ident_f)
                    nc.scalar.copy(out=PT[:, ts(mc, P)], in_=tp)

                # output_one = (P @ memory) * recip
                o1_psum = o1ps.tile([P, D], F32, tag="o1p")
                for mc in range(NMC):
                    for dc in range(ND2):
                        nc.tensor.matmul(o1_psum[:, ds(dc * DC, DC)],
                                         PT[:, ts(mc, P)],
                                         mem_big[:, mc, ds(dc * DC, DC)],
                                         start=(mc == 0), stop=(mc == NMC - 1))
                o1_sb = o1pool.tile([P, D], F32, tag="o1")
                for dc in range(ND2):
                    nc.vector.tensor_scalar_mul(out=o1_sb[:, ds(dc * DC, DC)],
                                                in0=o1_psum[:, ds(dc * DC, DC)],
                                                scalar1=recip)

                # write blocks 0..2
                nc.sync.dma_start(out=out_ap[b, lsl, 0:D], in_=_f(in_t))
                nc.sync.dma_start(out=out_ap[b, lsl, D:2 * D], in_=o1_sb)
                blk3 = b3pool.tile([P, D], F32, tag="b3")
                nc.vector.tensor_mul(out=blk3, in0=_f(in_t), in1=o1_sb)
                nc.sync.dma_start(out=out_ap[b, lsl, 2 * D:3 * D], in_=blk3)

            # --- finalize output_two: o2 = o2_raw / sum(exp(rowmax-30)) ---
            evall = smalls.tile([P, NLT], F32, tag="evall")
            nc.scalar.activation(out=evall, in_=nrm_all, func=Exp, scale=-1.0,
                                 bias=neg30)
            colsum_ps = attps.tile([NLT, 1], F32, tag="att")
            nc.tensor.matmul(colsum_ps, evall, ones_f, start=True, stop=True)
            cs_sb = smalls.tile([NLT, 1], F32, tag="cssb")
            nc.vector.tensor_copy(out=cs_sb, in_=colsum_ps)
            z2_ps = attps.tile([1, 1], F32, tag="att")
            nc.tensor.matmul(z2_ps, cs_sb, ones_f[0:NLT, 0:1], start=True,
                             stop=True)
            z2r = smalls.tile([1, 1], F32, tag="z2r")
            nc.vector.reciprocal(z2r, z2_ps)
            o2_sb = residents.tile([1, D], F32, tag="o2sb")
            nc.scalar.activation(out=o2_sb, in_=o2_ps, func=Copy, bias=0.0,
                                 scale=z2r)
            o2_dram = drampool.tile([1, D], F32, tag="o2d")
            nc.sync.dma_start(out=o2_dram, in_=o2_sb)
            o2b = residents.tile([P, D], F32, tag="o2b")
            nc.sync.dma_start(
                out=o2b,
                in_=bass.AP(tensor=o2_dram.tensor, offset=o2_dram.offset,
                            ap=[[0, P]] + list(o2_dram.ap)[1:]))

            # --- block 4 sweep: out[...,3D:4D] = o2 * o1 (re-read block 1) ---
            for lt in range(NLT):
                lsl = ts(lt, P)
                t4 = o1pool.tile([P, D], F32, tag="o1")
                nc.sync.dma_start(out=t4, in_=out_ap[b, lsl, D:2 * D])
                nc.vector.tensor_mul(out=t4, in0=t4, in1=o2b)
                nc.sync.dma_start(out=out_ap[b, lsl, 3 * D:4 * D], in_=t4)


def build_module(BPC, L, D, M, enable_asserts=False):
    nc = bacc.Bacc("TRN2", target_bir_lowering=False, debug=False,
                   enable_asserts=enable_asserts, num_devices=1)
    inp = nc.dram_tensor("input", (BPC, L, D), F32, kind="ExternalInput").ap()
    mem = nc.dram_tensor("memory", (BPC, M, D), F32, kind="ExternalInput").ap()
    msk = nc.dram_tensor("mask", (BPC, L), U8, kind="ExternalInput").ap()
    w_in = nc.dram_tensor("w_input", (D,), F32, kind="ExternalInput").ap()
    w_mem = nc.dram_tensor("w_memory", (D,), F32, kind="ExternalInput").ap()
    dsc = nc.dram_tensor("dot_scale", (D,), F32, kind="ExternalInput").ap()
    out = nc.dram_tensor("out", (BPC, L, 4 * D), F32, kind="ExternalOutput").ap()
    with tile.TileContext(nc) as tc:
        biattn_tile_kernel(tc, out, inp, mem, msk, w_in, w_mem, dsc,
                           BPC, L, D, M)
    nc.compile()
    return nc


_NC_CACHE = {}


def kernel(input, memory, mask, w_input, w_memory, dot_scale, trace=False):
    B, L, D = input.shape
    M = memory.shape[1]
    NCORES = 8
    BPC = B // NCORES
    key = (BPC, L, D, M)
    if key not in _NC_CACHE:
        _NC_CACHE[key] = build_module(*key)
    nc = _NC_CACHE[key]

    input = np.ascontiguousarray(np.asarray(input, dtype=np.float32))
    memory = np.ascontiguousarray(np.asarray(memory, dtype=np.float32))
    mask_u8 = np.ascontiguousarray(np.asarray(mask).astype(np.uint8))
    w_input = np.ascontiguousarray(np.asarray(w_input, dtype=np.float32))
    w_memory = np.ascontiguousarray(np.asarray(w_memory, dtype=np.float32))
    dot_scale = np.ascontiguousarray(np.asarray(dot_scale, dtype=np.float32))

    in_maps = []
    for c in range(NCORES):
        sl = slice(c * BPC, (c + 1) * BPC)
        in_maps.append({
            "input": input[sl], "memory": memory[sl], "mask": mask_u8[sl],
            "w_input": w_input, "w_memory": w_memory, "dot_scale": dot_scale,
        })
    res = run_bass_kernel_spmd(nc, in_maps, core_ids=list(range(NCORES)),
                               trace=trace)
    out = np.concatenate([res.results[c]["out"] for c in range(NCORES)], axis=0)
    if trace:
        kernel.last_exec_time_ns = res.exec_time_ns
        kernel.last_results = res
    return out

